# revision 1
# baseline (speedup 1.0000x reference)
"""GTA3Layer Trainium2 kernel (v2).

Sharding: 8 cores = 2 batches x 4 query-blocks of 512 rows. Each core
computes its 512 output rows end-to-end (attention over all 8 heads +
full key range, then Wo/residual/LN/FFN/LN); no collectives. The host
scatters inputs and gathers the per-core outputs.

Math notes (vs the jax reference):
  - softmax(s/4) followed by phi (pow(alpha,A) mask + L1 renorm) fuses:
    the softmax denominator cancels, so att = f*exp(s/4) / sum_m f*exp(s/4)
    with f = pow(alpha+1e-10, A) * (A != 0). Logits are O(1), so exp
    without max-subtraction is safe.
  - key-padding is folded into V and the denominator ones-column,
    query-padding into the per-row reciprocal -> no -inf masking.
  - the k-projection bias shifts all logits of a (q,head) row equally
    and cancels in the normalization -> dropped.
  - row-sums ride the att@v matmul as a 17th "ones" column of V.

Layout: transposed ([d, n] / [m, q]) throughout so the PE contracts over
partitions and LN stats come from ones-matmuls. Per-head 16-dim slices
sit on 32-partition strips so four heads' K=16 matmuls pack into the PE
array concurrently via tile_position. Logit/projection matmuls use
float32r (full-rate fp32); attention weights and V run in fp16.
"""

import numpy as np
from contextlib import ExitStack

B, N, D, H, DH = 2, 2048, 128, 8, 16
NCORES = 8
QB = 512          # query rows per core
NT = N // 128     # 16 key tiles


def _stripe_cols(W):
    outs = []
    for g in range(2):
        o = np.zeros((D, D), np.float32)
        for s in range(4):
            h = 4 * g + s
            o[:, 32 * s:32 * s + 16] = W[:, 16 * h:16 * h + 16]
        outs.append(o)
    return outs


def _stripe_rows(W):
    outs = []
    for g in range(2):
        o = np.zeros((D, D), np.float32)
        for s in range(4):
            h = 4 * g + s
            o[32 * s:32 * s + 16, :] = W[16 * h:16 * h + 16, :]
        outs.append(o)
    return outs


def _stripe_vec(b):
    outs = []
    for g in range(2):
        o = np.zeros((D, 1), np.float32)
        for s in range(4):
            h = 4 * g + s
            o[32 * s:32 * s + 16, 0] = b[16 * h:16 * h + 16]
        outs.append(o)
    return outs


def _build_program(repeat=1):
    import concourse.bacc as bacc
    import concourse.tile as tile
    import concourse.mybir as mybir

    f32 = mybir.dt.float32
    f32r = mybir.dt.float32r
    f16 = mybir.dt.float16
    AF = mybir.ActivationFunctionType
    OP = mybir.AluOpType

    nc = bacc.Bacc(trn_type="TRN2")

    def din(name, shape, dtype=f32):
        return nc.dram_tensor(name, shape, dtype, kind="ExternalInput")

    hT_d = din("hT", [D, N], f32r)
    hTq_d = din("hTq", [D, QB], f32r)
    AT_d = din("AT", [128, NT * QB], f16)
    WqA_d = din("WqA", [D, D], f32r); WqB_d = din("WqB", [D, D], f32r)
    bqA_d = din("bqA", [D, 1]); bqB_d = din("bqB", [D, 1])
    WkA_d = din("WkA", [D, D], f32r); WkB_d = din("WkB", [D, D], f32r)
    Wv_d = din("Wv", [D, D], f32r)
    bvk_d = din("bvk", [128, NT, D])
    kmm_d = din("kmm", [128, NT])
    qm_d = din("qm", [1, QB])
    lnc_d = din("lnc", [D, 1])
    WoA_d = din("WoA", [D, D], f32r); WoB_d = din("WoB", [D, D], f32r)
    boc_d = din("boc", [D, 1])
    W1_d = din("W1", [D, 2 * D])
    b1c_d = din("b1c", [D, 2])
    W2a_d = din("W2a", [D, D]); W2b_d = din("W2b", [D, D])
    b2c_d = din("b2c", [D, 1])
    g1c_d = din("g1c", [D, 1]); be1c_d = din("be1c", [D, 1])
    g2c_d = din("g2c", [D, 1]); be2c_d = din("be2c", [D, 1])
    xout_d = nc.dram_tensor("xout", [D, QB], f32, kind="ExternalOutput")

    with tile.TileContext(nc) as tc, ExitStack() as ctx:
        const = ctx.enter_context(tc.tile_pool(name="const", bufs=1))
        big = ctx.enter_context(tc.tile_pool(name="big", bufs=1))
        work = ctx.enter_context(tc.tile_pool(name="work", bufs=3))
        rows = ctx.enter_context(tc.tile_pool(name="rows", bufs=4))
        ps_s = ctx.enter_context(tc.tile_pool(name="ps_s", bufs=2, space="PSUM"))
        ps_o = ctx.enter_context(tc.tile_pool(name="ps_o", bufs=2, space="PSUM"))
        ps_m = ctx.enter_context(tc.tile_pool(name="ps_m", bufs=2, space="PSUM"))

        def load(pool, dram, tag):
            t = pool.tile(list(dram.shape), dram.dtype, tag=tag)
            nc.sync.dma_start(out=t[:], in_=dram[:])
            return t

        hT = load(big, hT_d, "hT")
        hTq = load(big, hTq_d, "hTq")
        AT = load(big, AT_d, "AT")
        Wq = [load(const, WqA_d, "WqA"), load(const, WqB_d, "WqB")]
        bq = [load(const, bqA_d, "bqA"), load(const, bqB_d, "bqB")]
        Wk = [load(const, WkA_d, "WkA"), load(const, WkB_d, "WkB")]
        Wv = load(const, Wv_d, "Wv")
        bvk = load(const, bvk_d, "bvk")
        kmm = load(const, kmm_d, "kmm")
        qm = load(const, qm_d, "qm")
        lnc = load(const, lnc_d, "lnc")
        Wo = [load(const, WoA_d, "WoA"), load(const, WoB_d, "WoB")]
        boc = load(const, boc_d, "boc")
        W1 = load(const, W1_d, "W1")
        b1c = load(const, b1c_d, "b1c")
        W2a = load(const, W2a_d, "W2a")
        W2b = load(const, W2b_d, "W2b")
        b2c = load(const, b2c_d, "b2c")
        g1c = load(const, g1c_d, "g1c")
        be1c = load(const, be1c_d, "be1c")
        g2c = load(const, g2c_d, "g2c")
        be2c = load(const, be2c_d, "be2c")

        ones = const.tile([128, 1], f32, tag="ones")
        nc.vector.memset(ones[:], 1.0)
        eps = const.tile([1, 1], f32, tag="eps")
        nc.vector.memset(eps[:], 1e-5)

        for _rep in range(repeat):
            # ---- q/k/v projections (transposed layouts) ----
            qT = []
            for g in range(2):
                p = ps_m.tile([128, QB], f32, tag="misc")
                nc.tensor.matmul(p[:], lhsT=Wq[g][:], rhs=hTq[:],
                                 start=True, stop=True)
                t = big.tile([128, QB], f32r, tag=f"qT{g}")
                nc.scalar.activation(t[:], p[:], AF.Identity, bias=bq[g][:, 0:1])
                qT.append(t)

            kT = []
            for g in range(2):
                t = big.tile([128, N], f32r, tag=f"kT{g}")
                for u in range(N // 512):
                    p = ps_m.tile([128, 512], f32, tag="misc")
                    nc.tensor.matmul(
                        p[:], lhsT=Wk[g][:], rhs=hT[:, 512 * u:512 * (u + 1)],
                        start=True, stop=True)
                    nc.vector.tensor_copy(t[:, 512 * u:512 * (u + 1)], p[:])
                kT.append(t)

            # v in [key, d] layout, per-head 17-wide slots (16 dims + ones
            # col = key mask), fp16
            vS = big.tile([128, NT, H, 17], f16, tag="vS")
            for j in range(NT):
                p = ps_m.tile([128, D], f32, tag="misc")
                nc.tensor.matmul(
                    p[:], lhsT=hT[:, 128 * j:128 * (j + 1)], rhs=Wv[:],
                    start=True, stop=True)
                nc.vector.scalar_tensor_tensor(
                    out=vS[:, j, :, 0:16],
                    in0=p.rearrange("p (h e) -> p h e", h=H),
                    scalar=kmm[:, j:j + 1],
                    in1=bvk[:, j, :].rearrange("p (h e) -> p h e", h=H),
                    op0=OP.mult, op1=OP.add)
            # all NT ones-columns (key mask) in one strided copy
            nc.vector.tensor_copy(
                vS[:, :, :, 16:17],
                kmm[:, :, None, None].to_broadcast([128, NT, H, 1]))

            # ---- phi factors fT[m, q] = pow(c, A^T) * (A != 0), fp16 ----
            # AT is host-packed as [128, NT*512]: AT[p, j*512+q] = A[q, 128j+p]
            fT = big.tile([128, NT * QB], f16, tag="fT")
            fe = big.tile([128, NT * QB], f16, tag="fe")
            nc.scalar.activation(fe[:], AT[:], AF.Exp, scale=lnc[:, 0:1])
            nc.vector.scalar_tensor_tensor(
                out=fT[:], in0=AT[:], scalar=1.0, in1=fe[:],
                op0=OP.min, op1=OP.mult)

            # ---- attention: j-pairs, 4-head strip packing ----
            oT = []
            for g in range(2):
                pso = ps_o.tile([128, QB], f32, tag="pso")
                for jp in range(NT // 2):
                    for s in range(4):
                        pss = ps_s.tile([128, 1024], f32, tag="pss")
                        for u in range(2):
                            j = 2 * jp + u
                            nc.tensor.matmul(
                                pss[:, 512 * u:512 * (u + 1)],
                                lhsT=kT[g][32 * s:32 * s + 16,
                                           128 * j:128 * (j + 1)],
                                rhs=qT[g][32 * s:32 * s + 16, :],
                                start=True, stop=True,
                                tile_position=(32 * s, 0))
                        w = work.tile([128, 1024], f16, tag="w")
                        nc.scalar.activation(w[:], pss[:], AF.Exp, scale=0.25)
                        wm = work.tile([128, 1024], f16, tag="wm")
                        nc.vector.tensor_mul(
                            wm[:], w[:],
                            fT[:, 1024 * jp:1024 * (jp + 1)])
                        for u in range(2):
                            j = 2 * jp + u
                            nc.tensor.matmul(
                                pso[32 * s:32 * s + 17, :],
                                lhsT=vS[:, j, 4 * g + s, :],
                                rhs=wm[:, 512 * u:512 * (u + 1)],
                                start=(j == 0), stop=(j == NT - 1),
                                tile_position=(0, 32 * s),
                                skip_group_check=True)

                # normalize: out_head[d, q] * qmask[q] / rowsum[q]
                ot = big.tile([128, QB], f32r, tag=f"oT{g}")
                bc = big.tile([128, QB], f32, tag=f"bc{g}")
                t128 = work.tile([128, QB], f32, tag="t128")
                for s in range(4):
                    nc.vector.tensor_copy(t128[32 * s:32 * s + 17, :],
                                          pso[32 * s:32 * s + 17, :])
                for s in range(4):
                    r0 = rows.tile([1, QB], f32, tag="r0")
                    nc.sync.dma_start(out=r0[:],
                                      in_=t128[32 * s + 16:32 * s + 17, :])
                    r1 = rows.tile([1, QB], f32, tag="r1")
                    nc.vector.tensor_scalar_add(r1[:], r0[:], 1e-30)
                    r2 = rows.tile([1, QB], f32, tag="r2")
                    nc.vector.reciprocal(r2[:], r1[:])
                    r3 = rows.tile([1, QB], f32, tag="r3")
                    nc.vector.tensor_mul(r3[:], r2[:], qm[:])
                    nc.sync.dma_start(
                        out=bc[32 * s:32 * s + 16, :],
                        in_=r3[:, None, :].to_broadcast([1, 16, QB]))
                    nc.vector.tensor_mul(
                        ot[32 * s:32 * s + 16, :],
                        t128[32 * s:32 * s + 16, :],
                        bc[32 * s:32 * s + 16, :])
                oT.append(ot)

            # ---- output projection + residual ----
            psy = ps_m.tile([128, QB], f32, tag="misc")
            k8 = 0
            for g in range(2):
                for s in range(4):
                    nc.tensor.matmul(
                        psy[:],
                        lhsT=Wo[g][32 * s:32 * s + 16, :],
                        rhs=oT[g][32 * s:32 * s + 16, :],
                        start=(k8 == 0), stop=(k8 == 7),
                        tile_position=(32 * s, 0))
                    k8 += 1

            xx1 = big.tile([128, 2, QB], f32, tag="xx1")
            nc.vector.scalar_tensor_tensor(
                out=xx1[:, 0, :], in0=psy[:], scalar=boc[:, 0:1], in1=hTq[:],
                op0=OP.add, op1=OP.add)

            def layernorm(xpair, gcol, becol, out_ap, out_dtype_note=None):
                nc.scalar.activation(xpair[:, 1, :], xpair[:, 0, :], AF.Square)
                sr = ps_m.tile([1, QB], f32, tag="misc")
                nc.tensor.matmul(sr[:], lhsT=ones[:], rhs=xpair[:, 0, :],
                                 start=True, stop=True)
                qr = ps_m.tile([1, QB], f32, tag="misc")
                nc.tensor.matmul(qr[:], lhsT=ones[:], rhs=xpair[:, 1, :],
                                 start=True, stop=True)
                mu = rows.tile([1, QB], f32, tag="r1")
                nc.vector.tensor_scalar_mul(mu[:], sr[:], 1.0 / D)
                musq = rows.tile([1, QB], f32, tag="r2")
                nc.vector.tensor_mul(musq[:], mu[:], mu[:])
                var = rows.tile([1, QB], f32, tag="r3")
                nc.vector.scalar_tensor_tensor(
                    out=var[:], in0=qr[:], scalar=1.0 / D, in1=musq[:],
                    op0=OP.mult, op1=OP.subtract)
                sd = rows.tile([1, QB], f32, tag="r4")
                nc.scalar.activation(sd[:], var[:], AF.Sqrt, bias=eps[:, 0:1])
                rstd = rows.tile([1, QB], f32, tag="r5")
                nc.vector.reciprocal(rstd[:], sd[:])
                mub = work.tile([128, QB], f32, tag="mub")
                nc.sync.dma_start(
                    out=mub[:], in_=mu[:, None, :].to_broadcast([1, 128, QB]))
                rsb = work.tile([128, QB], f32, tag="rsb")
                nc.sync.dma_start(
                    out=rsb[:], in_=rstd[:, None, :].to_broadcast([1, 128, QB]))
                t1 = work.tile([128, QB], f32, tag="t1")
                nc.vector.tensor_sub(t1[:], xpair[:, 0, :], mub[:])
                nc.vector.tensor_mul(t1[:], t1[:], rsb[:])
                nc.scalar.activation(out_ap, t1[:], AF.Identity,
                                     bias=becol[:, 0:1], scale=gcol[:, 0:1])

            x2 = big.tile([128, QB], f32, tag="x2")
            layernorm(xx1, g1c, be1c, x2[:])

            # ---- FFN ----
            f1 = []
            for u in range(2):
                p = ps_m.tile([128, QB], f32, tag="misc")
                nc.tensor.matmul(p[:], lhsT=W1[:, 128 * u:128 * (u + 1)],
                                 rhs=x2[:], start=True, stop=True)
                t = big.tile([128, QB], f32, tag=f"f1{u}")
                nc.scalar.activation(t[:], p[:], AF.Relu, bias=b1c[:, u:u + 1])
                f1.append(t)
            psy2 = ps_m.tile([128, QB], f32, tag="misc")
            nc.tensor.matmul(psy2[:], lhsT=W2a[:], rhs=f1[0][:],
                             start=True, stop=False)
            nc.tensor.matmul(psy2[:], lhsT=W2b[:], rhs=f1[1][:],
                             start=False, stop=True)

            xx2 = big.tile([128, 2, QB], f32, tag="xx2")
            nc.vector.scalar_tensor_tensor(
                out=xx2[:, 0, :], in0=psy2[:], scalar=b2c[:, 0:1], in1=x2[:],
                op0=OP.add, op1=OP.add)

            xf = big.tile([128, QB], f32, tag="xf")
            layernorm(xx2, g2c, be2c, xf[:])
            nc.sync.dma_start(out=xout_d[:], in_=xf[:])

    return nc


def _host_inputs(h, A, lengths, alpha, Wq, bq, Wk, bk, Wv, bv, Wo, bo,
                 W1, b1, W2, b2, g1, be1, g2, be2):
    """Build the 8 per-core input maps."""
    h = np.asarray(h, np.float32)
    A = np.asarray(A)
    lengths = np.asarray(lengths)
    al = max(float(np.float32(alpha)), 0.0)
    lnc_val = np.float32(np.log(np.float32(al) + np.float32(1e-10)))

    WqS = _stripe_cols(np.asarray(Wq, np.float32))
    bqS = _stripe_vec(np.asarray(bq, np.float32))
    WkS = _stripe_cols(np.asarray(Wk, np.float32))
    WoS = _stripe_rows(np.asarray(Wo, np.float32))
    Wv32 = np.ascontiguousarray(np.asarray(Wv, np.float32))
    W1a = np.ascontiguousarray(np.asarray(W1, np.float32))
    W2_ = np.asarray(W2, np.float32)
    b1_ = np.asarray(b1, np.float32)

    common = dict(
        WqA=WqS[0], WqB=WqS[1], bqA=bqS[0], bqB=bqS[1],
        WkA=WkS[0], WkB=WkS[1], Wv=Wv32,
        lnc=np.full((D, 1), lnc_val, np.float32),
        WoA=np.ascontiguousarray(WoS[0]), WoB=np.ascontiguousarray(WoS[1]),
        boc=np.asarray(bo, np.float32).reshape(D, 1).copy(),
        W1=W1a,
        b1c=np.ascontiguousarray(b1_.reshape(2, D).T),
        W2a=np.ascontiguousarray(W2_[:D]), W2b=np.ascontiguousarray(W2_[D:]),
        b2c=np.asarray(b2, np.float32).reshape(D, 1).copy(),
        g1c=np.asarray(g1, np.float32).reshape(D, 1).copy(),
        be1c=np.asarray(be1, np.float32).reshape(D, 1).copy(),
        g2c=np.asarray(g2, np.float32).reshape(D, 1).copy(),
        be2c=np.asarray(be2, np.float32).reshape(D, 1).copy(),
    )

    in_maps = []
    for c in range(NCORES):
        b = c // 4
        q0 = (c % 4) * QB
        L = int(lengths[b])
        kmask = (np.arange(N) < L).astype(np.float32)
        m = dict(common)
        m["hT"] = np.ascontiguousarray(h[b].T)
        m["hTq"] = np.ascontiguousarray(h[b, q0:q0 + QB].T)
        # AT[p, j*512+q] = A[b, q0+q, 128j+p]
        at = A[b, q0:q0 + QB, :].astype(np.float16)          # [QB, N]
        at = at.T.reshape(NT, 128, QB).transpose(1, 0, 2)     # [128, NT, QB]
        m["AT"] = np.ascontiguousarray(at.reshape(128, NT * QB))
        m["bvk"] = np.ascontiguousarray(
            (np.asarray(bv, np.float32)[None, :] * kmask[:, None])
            .reshape(NT, 128, D).transpose(1, 0, 2))
        m["kmm"] = np.ascontiguousarray(kmask.reshape(NT, 128).T)
        m["qm"] = ((np.arange(q0, q0 + QB) < L)
                   .astype(np.float32).reshape(1, QB))
        in_maps.append(m)
    return in_maps


_CACHE = {}
TRACE = False


def kernel(**inputs):
    import os
    from concourse.bass_utils import run_bass_kernel_spmd

    # The NTFF trace path needs antenv.axon_hooks, absent in this
    # container — make sure an inherited BASS_TRACE can't select it.
    os.environ["BASS_NEVER_TRACE"] = "1"
    in_maps = _host_inputs(**inputs)
    if "nc" not in _CACHE:
        nc = _build_program()
        nc.finalize()
        _CACHE["nc"] = nc
    nc = _CACHE["nc"]
    res = run_bass_kernel_spmd(nc, in_maps, core_ids=list(range(NCORES)),
                               trace=TRACE)
    _CACHE["last"] = res
    out = np.empty((B, N, D), np.float32)
    for c in range(NCORES):
        b = c // 4
        q0 = (c % 4) * QB
        out[b, q0:q0 + QB, :] = res.results[c]["xout"].T
    return out

